# revision 5
# baseline (speedup 1.0000x reference)
"""Trainium2 Bass kernel for nn_HANGraphClassifier.

Because every node of a type shares one embedding, the GAT attention collapses
analytically: per-edge softmax weights become 1/deg and each dst node's
aggregated message is src_type_vec * (in_degree > 0). The whole forward pass
therefore reduces to per-batch counts of dst nodes with >=1 incoming edge
(per edge type, plus the joint fp&sp combination for proc nodes), followed by
tiny [BSZ,64] parameter-only math.

Device work (the O(E)+O(N) part): presence-mask scatter over 4.8M edges and
per-batch counting, on 8 NeuronCores.

Sharding (per the hint, graph/data-parallel by destination-node partition):
 - batches 16c..16c+15 -> core c (batch arrays are sorted, so each core owns a
   contiguous dst-node range per node type).
 - within a core, Q7 group g (16 SBUF partitions) owns the node range of
   batches (16c+2g, 16c+2g+1) -- a "bucket" of ~1560 nodes (<= 2046).
 - each edge type's dst list is routed on the host into these 64 buckets and
   converted to bucket-local int16 indices (standard global->local id
   conversion during partitioning); a bucket's edges are split arbitrarily
   across its 16 partitions.

Device program per core (single SPMD program, ~30 instructions):
 1. DMA the routed [128, Ktot] int16 index array in.
 2. gpsimd.local_scatter per edge type: each partition scatters 1.0 into its
    own [2046] bf16 table copy (SuperGather HW; duplicates all write 1.0).
 3. PE matmul with a [128->8] group-indicator weight: sums the 16 copies of
    each group -> PSUM [32, 2046] per-(type,group) copy-counts.
 4. DVE: presence = min(count,1); joint = min(pres_fp, pres_sp);
    multiply by a host-built segment mask (1.0 for the bucket's first batch,
    4096.0 for the second) and reduce -> [40,1] encoded per-batch counts.
 5. DMA counts out; host decodes c0 = v % 4096, c1 = v // 4096.
"""

import os

import numpy as np

N_PROC, N_FILE, N_SOCK = 100000, 100000, 50000
H, D, HID, BSZ, NCLS = 4, 16, 64, 128, 2
NCORE = 8
BPC = BSZ // NCORE          # batches per core = 16
NGRP = 8                    # Q7 groups per core
TBL = 2046                  # local_scatter table entries (limit: n*32 < 2^16)
NROW = 40                   # 4 types * 8 groups + 8 joint rows
F32 = np.float32


def _batch_starts(batch, n_nodes):
    s = np.searchsorted(batch, np.arange(BSZ + 1)).astype(np.int64)
    assert s[-1] == n_nodes
    return s


def _route_edges(dst, starts):
    """Route one edge type's dst list into 64 batch-pair buckets.

    Returns ([64,16,K] int16 local idx array padded with -1, K)."""
    bounds = starts[::2]  # [65] node-range bounds of the 64 buckets
    bid = (np.searchsorted(bounds, dst, side="right") - 1).astype(np.int32)
    order = np.argsort(bid, kind="stable")
    sd = dst[order]
    cnts = np.bincount(bid, minlength=64)
    loc = (sd - np.repeat(bounds[:64], cnts)).astype(np.int16)
    per_part = (cnts + 15) // 16
    K = int(max(2, per_part.max()))
    K += K % 2  # num_idxs must be even
    arr = np.full((64, 16 * K), -1, np.int16)
    off = np.concatenate([[0], np.cumsum(cnts)])
    for k in range(64):
        if cnts[k]:
            arr[k, : cnts[k]] = loc[off[k] : off[k] + cnts[k]]
    return arr.reshape(64, 16, K), K


def _host_counts(dst, batch, n_nodes):
    m = np.zeros(n_nodes, F32)
    m[dst] = 1.0
    return m, np.bincount(batch, weights=m, minlength=BSZ).astype(F32)


def _epilogue(inp, c_pf, c_fp, c_ps, c_sp, c_11, cnt_p, cnt_f, cnt_s):
    """Tiny parameter-only math reproducing the collapsed reference."""
    node_emb, proj_w, proj_b = inp["node_emb"], inp["proj_w"], inp["proj_b"]
    k_w, k_b, q_vec = inp["k_w"], inp["k_b"], inp["q_vec"]
    p = [node_emb[i] @ proj_w[i].T + proj_b[i] for i in range(3)]
    rp = [np.maximum(v, 0).astype(F32) for v in p]

    def score(v, n1, N):
        t1 = np.tanh(v @ k_w.T + k_b)
        t0 = np.tanh(k_b)
        mean = (n1 * t1 + (N - n1) * t0) / F32(N)
        return (q_vec * mean).sum()

    s1 = score(rp[1], c_fp.sum(), N_PROC)
    s2 = score(rp[2], c_sp.sum(), N_PROC)
    e = np.exp(np.array([s1, s2]) - max(s1, s2))
    attn = (e / e.sum()).astype(F32)

    h10 = np.maximum(attn[0] * rp[1], 0)
    h01 = np.maximum(attn[1] * rp[2], 0)
    h11 = np.maximum(attn[0] * rp[1] + attn[1] * rp[2], 0)

    c_10, c_01 = c_fp - c_11, c_sp - c_11
    pool_p = (np.outer(c_10, h10) + np.outer(c_01, h01) + np.outer(c_11, h11)) \
        / np.maximum(cnt_p, 1.0)[:, None]
    pool_f = np.outer(c_pf, rp[0]) / np.maximum(cnt_f, 1.0)[:, None]
    pool_s = np.outer(c_ps, rp[0]) / np.maximum(cnt_s, 1.0)[:, None]
    g = ((pool_p + pool_f + pool_s) / 3.0).astype(F32)
    h = np.maximum(g @ inp["cls_w1"].T + inp["cls_b1"], 0)
    return (h @ inp["cls_w2"].T + inp["cls_b2"]).astype(F32)


_PROG_CACHE = {}


def _build_program(Ks):
    import concourse.bacc as bacc
    import concourse.mybir as mybir
    import concourse.tile as tile

    key = tuple(Ks)
    if key in _PROG_CACHE:
        return _PROG_CACHE[key]

    Ktot = sum(Ks)
    Kmax = max(Ks)
    nc = bacc.Bacc("TRN2", target_bir_lowering=False, debug=False)
    ed_d = nc.dram_tensor("edges", [128, Ktot], mybir.dt.int16, kind="ExternalInput")
    wm_d = nc.dram_tensor("wmat", [128, 512], mybir.dt.bfloat16, kind="ExternalInput")
    w2_d = nc.dram_tensor("wmat2", [128, 128], mybir.dt.bfloat16, kind="ExternalInput")
    m1_d = nc.dram_tensor("mask1", [128, TBL], mybir.dt.bfloat16, kind="ExternalInput")
    m2_d = nc.dram_tensor("mask2", [128, TBL], mybir.dt.bfloat16, kind="ExternalInput")
    ct_d = nc.dram_tensor("counts", [128, 2], mybir.dt.float32, kind="ExternalOutput")

    with tile.TileContext(nc, trace_sim=False) as tc:
        with (
            tc.tile_pool(name="sb", bufs=1) as pool,
            tc.tile_pool(name="ps", bufs=1, space="PSUM") as ppool,
        ):
            ed = pool.tile([128, Ktot], mybir.dt.int16)
            wm = pool.tile([128, 512], mybir.dt.bfloat16)
            w2 = pool.tile([128, 128], mybir.dt.bfloat16)
            m1 = pool.tile([128, TBL], mybir.dt.bfloat16)
            m2 = pool.tile([128, TBL], mybir.dt.bfloat16)
            ones = pool.tile([128, Kmax], mybir.dt.bfloat16)
            tbl = pool.tile([128, 4 * TBL], mybir.dt.bfloat16)
            pres = pool.tile([128, TBL], mybir.dt.bfloat16)
            pres2 = pool.tile([128, TBL], mybir.dt.bfloat16)
            prod = pool.tile([128, TBL], mybir.dt.bfloat16)
            prod2 = pool.tile([128, TBL], mybir.dt.bfloat16)
            red = pool.tile([128, 2], mybir.dt.float32)
            ps = ppool.tile([128, TBL], mybir.dt.float32)
            ps2 = ppool.tile([128, TBL], mybir.dt.float32)

            nc.sync.dma_start(ed[:], ed_d[:])
            nc.sync.dma_start(wm[:], wm_d[:])
            nc.sync.dma_start(w2[:], w2_d[:])
            nc.sync.dma_start(m1[:], m1_d[:])
            nc.sync.dma_start(m2[:], m2_d[:])
            nc.vector.memset(ones[:], 1.0)

            ofs = 0
            for t in range(4):
                nc.gpsimd.local_scatter(
                    tbl[:, t * TBL : (t + 1) * TBL],
                    ones[:, : Ks[t]],
                    ed[:, ofs : ofs + Ks[t]],
                    channels=128,
                    num_elems=TBL,
                    num_idxs=Ks[t],
                )
                ofs += Ks[t]

            # Stage 1: per-(type,group) copy-count sums. Type t's [128,128]
            # one-hot weight places group g's sum at out partition 32t+g and
            # zeros elsewhere, so all four types accumulate into one PSUM
            # group per 512-chunk.
            for j0 in range(0, TBL, 512):
                j1 = min(j0 + 512, TBL)
                for t in range(4):
                    nc.tensor.matmul(
                        out=ps[:, j0:j1],
                        lhsT=wm[:, 128 * t : 128 * t + 128],
                        rhs=tbl[:, t * TBL + j0 : t * TBL + j1],
                        start=(t == 0),
                        stop=(t == 3),
                    )

            # presence = min(copies, 1)
            nc.vector.tensor_scalar(
                pres[:], ps[:], 1.0, None, op0=mybir.AluOpType.min
            )

            # Stage 2: joint fp&sp -- re-align fp (rows 32..39) and sp
            # (rows 96..103) onto partitions 0..7 by summing, then sum-1
            # clamped at 0 gives the AND.
            for j0 in range(0, TBL, 512):
                j1 = min(j0 + 512, TBL)
                nc.tensor.matmul(
                    out=ps2[:, j0:j1],
                    lhsT=w2[:],
                    rhs=pres[:, j0:j1],
                    start=True,
                    stop=True,
                )
            nc.vector.tensor_scalar(
                pres2[:], ps2[:], 1.0, 0.0,
                op0=mybir.AluOpType.subtract, op1=mybir.AluOpType.max,
            )

            nc.vector.tensor_tensor(
                out=prod[:], in0=pres[:], in1=m1[:], op=mybir.AluOpType.mult
            )
            nc.vector.tensor_reduce(
                out=red[:, 0:1], in_=prod[:], axis=mybir.AxisListType.X,
                op=mybir.AluOpType.add,
            )
            nc.vector.tensor_tensor(
                out=prod2[:], in0=pres2[:], in1=m2[:], op=mybir.AluOpType.mult
            )
            nc.vector.tensor_reduce(
                out=red[:, 1:2], in_=prod2[:], axis=mybir.AxisListType.X,
                op=mybir.AluOpType.add,
            )
            nc.sync.dma_start(ct_d[:], red[:])

    nc.compile()
    _PROG_CACHE[key] = nc
    return nc


def kernel(**inputs):
    import ml_dtypes

    inp = {k: np.asarray(v) for k, v in inputs.items()}
    bf16 = ml_dtypes.bfloat16

    starts_p = _batch_starts(inp["batch_proc"], N_PROC)
    starts_f = _batch_starts(inp["batch_file"], N_FILE)
    starts_s = _batch_starts(inp["batch_sock"], N_SOCK)
    cnt_p = np.diff(starts_p).astype(F32)
    cnt_f = np.diff(starts_f).astype(F32)
    cnt_s = np.diff(starts_s).astype(F32)

    # (dst array, node-type starts) per edge type; dst node spaces:
    # pf->file, fp->proc, ps->sock, sp->proc
    types = [
        (inp["ei_pf_dst"], starts_f),
        (inp["ei_fp_dst"], starts_p),
        (inp["ei_ps_dst"], starts_s),
        (inp["ei_sp_dst"], starts_p),
    ]

    # A bucket (= node range of 2 consecutive batches) must fit the 2046-entry
    # table. Statistically certain for the stated generator; otherwise fall
    # back to a host implementation so correctness is never at risk.
    ok = all(int(np.diff(s[::2]).max()) <= TBL for _, s in types)
    if not ok or os.environ.get("KERNEL_HOST_FALLBACK"):
        m_pf, c_pf = _host_counts(inp["ei_pf_dst"], inp["batch_file"], N_FILE)
        m_fp, c_fp = _host_counts(inp["ei_fp_dst"], inp["batch_proc"], N_PROC)
        m_ps, c_ps = _host_counts(inp["ei_ps_dst"], inp["batch_sock"], N_SOCK)
        m_sp, c_sp = _host_counts(inp["ei_sp_dst"], inp["batch_proc"], N_PROC)
        c_11 = np.bincount(inp["batch_proc"], weights=m_fp * m_sp,
                           minlength=BSZ).astype(F32)
        return _epilogue(inp, c_pf, c_fp, c_ps, c_sp, c_11, cnt_p, cnt_f, cnt_s)

    routed = []
    Ks = []
    for dst, s in types:
        arr, K = _route_edges(dst, s)
        routed.append(arr)
        Ks.append(K)

    # Shared weights: type t's one-hot [128,128] block places group g's
    # 16-copy sum at output partition 32t+g; wmat2 folds fp(32+g) + sp(96+g)
    # onto partition g for the joint AND.
    parts = np.arange(128)
    wmat = np.zeros((128, 512), bf16)
    for t in range(4):
        wmat[parts, 128 * t + 32 * t + parts // 16] = 1.0
    wmat2 = np.zeros((128, 128), bf16)
    g8 = np.arange(NGRP)
    wmat2[32 + g8, g8] = 1.0
    wmat2[96 + g8, g8] = 1.0

    in_maps = []
    for c in range(NCORE):
        edges = np.concatenate(
            [routed[t][8 * c : 8 * c + 8].reshape(128, Ks[t]) for t in range(4)],
            axis=1,
        )
        mask1 = np.zeros((128, TBL), bf16)
        mask2 = np.zeros((128, TBL), bf16)
        for t, (_, s) in enumerate(types):
            for g in range(NGRP):
                b0 = BPC * c + 2 * g
                n0 = int(s[b0 + 1] - s[b0])
                n1 = int(s[b0 + 2] - s[b0 + 1])
                mask1[32 * t + g, :n0] = 1.0
                mask1[32 * t + g, n0 : n0 + n1] = 4096.0
                if t == 1:  # proc segments also drive the joint rows
                    mask2[g, :n0] = 1.0
                    mask2[g, n0 : n0 + n1] = 4096.0
        in_maps.append({
            "edges": np.ascontiguousarray(edges),
            "wmat": wmat, "wmat2": wmat2, "mask1": mask1, "mask2": mask2,
        })

    nc = _build_program(Ks)
    from concourse.bass_utils import run_bass_kernel_spmd

    try:
        res = run_bass_kernel_spmd(
            nc, in_maps, core_ids=list(range(NCORE)),
            trace=bool(os.environ.get("KERNEL_TRACE")),
        )
    except ModuleNotFoundError:
        res = run_bass_kernel_spmd(
            nc, in_maps, core_ids=list(range(NCORE)), trace=False
        )
    if os.environ.get("KERNEL_TRACE"):
        kernel.last_results = res

    # Decode per-(type,group) encoded counts back to per-batch counts
    c_arr = np.zeros((5, BSZ), F32)  # pf, fp, ps, sp, joint
    for c in range(NCORE):
        v = res.results[c]["counts"].astype(np.int64)  # [128, 2]
        for g in range(NGRP):
            b0 = BPC * c + 2 * g
            for t in range(4):
                c_arr[t, b0] = v[32 * t + g, 0] % 4096
                c_arr[t, b0 + 1] = v[32 * t + g, 0] // 4096
            c_arr[4, b0] = v[g, 1] % 4096
            c_arr[4, b0 + 1] = v[g, 1] // 4096
    return _epilogue(inp, c_arr[0], c_arr[1], c_arr[2], c_arr[3], c_arr[4],
                     cnt_p, cnt_f, cnt_s)


# revision 11
# speedup vs baseline: 1.2063x; 1.2063x over previous
"""Trainium2 Bass kernel for nn_HANGraphClassifier.

Because every node of a type shares one embedding, the GAT attention collapses
analytically: per-edge softmax weights become 1/deg and each dst node's
aggregated message is src_type_vec * (in_degree > 0). The whole forward pass
therefore reduces to per-batch counts of dst nodes with >=1 incoming edge
(per edge type, plus the joint fp&sp combination for proc nodes), followed by
tiny [BSZ,64] parameter-only math.

Device work (the O(E)+O(N) part): presence-mask scatter over 4.8M edges and
per-batch counting, on 8 NeuronCores.

Sharding (per the hint, graph/data-parallel by destination-node partition):
 - batches 16c..16c+15 -> core c (batch arrays are sorted, so each core owns a
   contiguous dst-node range per node type).
 - within a core, Q7 group g (16 SBUF partitions) owns the node range of
   batches (16c+2g, 16c+2g+1) -- a "bucket" of ~1560 nodes (<= 2046).
 - each edge type's dst list is routed on the host into these 64 buckets and
   converted to bucket-local int16 indices (standard global->local id
   conversion during partitioning); a bucket's edges are split arbitrarily
   across its 16 partitions.

Device program per core (single SPMD program, ~30 instructions):
 1. DMA the routed [128, Ktot] int16 index array in.
 2. gpsimd.local_scatter per edge type: each partition scatters 1.0 into its
    own [2046] bf16 table copy (SuperGather HW; duplicates all write 1.0).
 3. PE matmul with a [128->8] group-indicator weight: sums the 16 copies of
    each group -> PSUM [32, 2046] per-(type,group) copy-counts.
 4. DVE: presence = min(count,1); joint = min(pres_fp, pres_sp);
    multiply by a host-built segment mask (1.0 for the bucket's first batch,
    4096.0 for the second) and reduce -> [40,1] encoded per-batch counts.
 5. DMA counts out; host decodes c0 = v % 4096, c1 = v // 4096.
"""

import os

import numpy as np

N_PROC, N_FILE, N_SOCK = 100000, 100000, 50000
H, D, HID, BSZ, NCLS = 4, 16, 64, 128, 2
NCORE = 8
BPC = BSZ // NCORE          # batches per core = 16
NGRP = 8                    # Q7 groups per core
TBL = 2046                  # local_scatter table entries (limit: n*32 < 2^16)
NROW = 40                   # 4 types * 8 groups + 8 joint rows
F32 = np.float32


def _batch_starts(batch, n_nodes):
    s = np.searchsorted(batch, np.arange(BSZ + 1)).astype(np.int64)
    assert s[-1] == n_nodes
    return s


def _route_edges(dst, starts, seg_off):
    """Route one edge type's dst list into 64 batch-pair buckets; local index
    = dst - batch_start, with the bucket's second batch placed at column
    seg_off so per-batch counts fall out of a fixed-stride reduce.

    Returns ([64,16,K] int16 local idx array padded with -1, K)."""
    bid = (np.searchsorted(starts, dst, side="right") - 1).astype(np.int32)
    order = np.argsort(bid, kind="stable")
    sd = dst[order]
    sb = bid[order]
    loc = (sd - starts[sb] + (sb & 1) * seg_off).astype(np.int16)
    cnts = np.bincount(bid >> 1, minlength=64)
    per_part = (cnts + 15) // 16
    K = int(max(2, per_part.max()))
    K += K % 2  # num_idxs must be even
    arr = np.full((64, 16 * K), -1, np.int16)
    off = np.concatenate([[0], np.cumsum(cnts)])
    for k in range(64):
        if cnts[k]:
            arr[k, : cnts[k]] = loc[off[k] : off[k] + cnts[k]]
    return arr.reshape(64, 16, K), K


def _host_counts(dst, batch, n_nodes):
    m = np.zeros(n_nodes, F32)
    m[dst] = 1.0
    return m, np.bincount(batch, weights=m, minlength=BSZ).astype(F32)


def _epilogue(inp, c_pf, c_fp, c_ps, c_sp, c_11, cnt_p, cnt_f, cnt_s):
    """Tiny parameter-only math reproducing the collapsed reference."""
    node_emb, proj_w, proj_b = inp["node_emb"], inp["proj_w"], inp["proj_b"]
    k_w, k_b, q_vec = inp["k_w"], inp["k_b"], inp["q_vec"]
    p = [node_emb[i] @ proj_w[i].T + proj_b[i] for i in range(3)]
    rp = [np.maximum(v, 0).astype(F32) for v in p]

    def score(v, n1, N):
        t1 = np.tanh(v @ k_w.T + k_b)
        t0 = np.tanh(k_b)
        mean = (n1 * t1 + (N - n1) * t0) / F32(N)
        return (q_vec * mean).sum()

    s1 = score(rp[1], c_fp.sum(), N_PROC)
    s2 = score(rp[2], c_sp.sum(), N_PROC)
    e = np.exp(np.array([s1, s2]) - max(s1, s2))
    attn = (e / e.sum()).astype(F32)

    h10 = np.maximum(attn[0] * rp[1], 0)
    h01 = np.maximum(attn[1] * rp[2], 0)
    h11 = np.maximum(attn[0] * rp[1] + attn[1] * rp[2], 0)

    c_10, c_01 = c_fp - c_11, c_sp - c_11
    pool_p = (np.outer(c_10, h10) + np.outer(c_01, h01) + np.outer(c_11, h11)) \
        / np.maximum(cnt_p, 1.0)[:, None]
    pool_f = np.outer(c_pf, rp[0]) / np.maximum(cnt_f, 1.0)[:, None]
    pool_s = np.outer(c_ps, rp[0]) / np.maximum(cnt_s, 1.0)[:, None]
    g = ((pool_p + pool_f + pool_s) / 3.0).astype(F32)
    h = np.maximum(g @ inp["cls_w1"].T + inp["cls_b1"], 0)
    return (h @ inp["cls_w2"].T + inp["cls_b2"]).astype(F32)


_PROG_CACHE = {}


def _build_program(Ks, offs):
    import concourse.bacc as bacc
    import concourse.mybir as mybir
    import concourse.tile as tile

    key = (tuple(Ks), tuple(offs))
    if key in _PROG_CACHE:
        return _PROG_CACHE[key]

    Ktot = sum(Ks)
    Kmax = max(Ks)
    elems = [2 * o for o in offs]         # per-type table size (2 segments)
    ecol = np.concatenate([[0], np.cumsum(elems)]).astype(int)
    emax = max(elems)
    ep = elems[1]                          # proc table width (fp & sp share)
    nc = bacc.Bacc("TRN2", target_bir_lowering=False, debug=False)
    ed_d = nc.dram_tensor("edges", [128, Ktot], mybir.dt.int16, kind="ExternalInput")
    wm_d = nc.dram_tensor("wmat", [128, 8], mybir.dt.bfloat16, kind="ExternalInput")
    w2_d = nc.dram_tensor("wmat2", [128, 128], mybir.dt.bfloat16, kind="ExternalInput")
    ct_d = nc.dram_tensor("counts", [128, 4], mybir.dt.float32, kind="ExternalOutput")

    with tile.TileContext(nc, trace_sim=False) as tc:
        with (
            tc.tile_pool(name="sb", bufs=1) as pool,
            tc.tile_pool(name="ps", bufs=1, space="PSUM") as ppool,
        ):
            ed = pool.tile([128, Ktot], mybir.dt.int16)
            wm = pool.tile([128, 8], mybir.dt.bfloat16)
            w2 = pool.tile([128, 128], mybir.dt.bfloat16)
            ones = pool.tile([128, Kmax], mybir.dt.bfloat16)
            tbl = pool.tile([128, int(ecol[4])], mybir.dt.bfloat16)
            pres = pool.tile([128, emax], mybir.dt.bfloat16)
            pres2 = pool.tile([128, ep], mybir.dt.bfloat16)
            red = pool.tile([128, 4], mybir.dt.float32)
            ps = ppool.tile([128, emax], mybir.dt.float32)
            ps2 = ppool.tile([128, ep], mybir.dt.float32)

            nc.sync.dma_start(ed[:], ed_d[:])
            nc.sync.dma_start(wm[:], wm_d[:])
            nc.sync.dma_start(w2[:], w2_d[:])
            nc.vector.memset(ones[:], 1.0)
            # stage-2 contracts over all 128 pres partitions; unused rows
            # must be 0.0, not stale SBUF (0 * NaN would poison PSUM)
            nc.vector.memset(pres[:], 0.0)

            ofs = 0
            for t in range(4):
                e0, e1 = int(ecol[t]), int(ecol[t + 1])
                nc.gpsimd.local_scatter(
                    tbl[:, e0:e1],
                    ones[:, : Ks[t]],
                    ed[:, ofs : ofs + Ks[t]],
                    channels=128,
                    num_elems=elems[t],
                    num_idxs=Ks[t],
                )
                ofs += Ks[t]
                # per-(type,group) copy-count sums land at partitions
                # 32t+g via explicit PE tile position; presence + per-batch
                # reduce for this type overlap the next type's scatter.
                for j0 in range(0, elems[t], 512):
                    j1 = min(j0 + 512, elems[t])
                    nc.tensor.matmul(
                        out=ps[32 * t : 32 * t + 8, j0:j1],
                        lhsT=wm[:, 0:8],
                        rhs=tbl[:, e0 + j0 : e0 + j1],
                        start=True,
                        stop=True,
                        tile_position=(0, 32 * t),
                    )
                nc.vector.tensor_scalar(
                    pres[32 * t : 32 * t + 8, : elems[t]],
                    ps[32 * t : 32 * t + 8, : elems[t]],
                    1.0, None, op0=mybir.AluOpType.min,
                )
                nc.vector.tensor_reduce(
                    out=red[32 * t : 32 * t + 8, 0:2],
                    in_=pres[32 * t : 32 * t + 8, : elems[t]].rearrange(
                        "p (s o) -> p s o", s=2
                    ),
                    axis=mybir.AxisListType.X,
                    op=mybir.AluOpType.add,
                )

            # joint fp&sp: re-align fp (rows 32..39) and sp (rows 96..103)
            # onto partitions 0..7 by summing; sum-1 clamped at 0 is the AND.
            for j0 in range(0, ep, 512):
                j1 = min(j0 + 512, ep)
                nc.tensor.matmul(
                    out=ps2[:, j0:j1],
                    lhsT=w2[:],
                    rhs=pres[:, j0:j1],
                    start=True,
                    stop=True,
                )
            nc.vector.tensor_scalar(
                pres2[:], ps2[:], 1.0, 0.0,
                op0=mybir.AluOpType.subtract, op1=mybir.AluOpType.max,
            )
            nc.vector.tensor_reduce(
                out=red[0:8, 2:4],
                in_=pres2[0:8, :].rearrange("p (s o) -> p s o", s=2),
                axis=mybir.AxisListType.X,
                op=mybir.AluOpType.add,
            )
            nc.sync.dma_start(ct_d[:], red[:])

    nc.compile()
    _PROG_CACHE[key] = nc
    return nc


def kernel(**inputs):
    import ml_dtypes

    inp = {k: np.asarray(v) for k, v in inputs.items()}
    bf16 = ml_dtypes.bfloat16

    starts_p = _batch_starts(inp["batch_proc"], N_PROC)
    starts_f = _batch_starts(inp["batch_file"], N_FILE)
    starts_s = _batch_starts(inp["batch_sock"], N_SOCK)
    cnt_p = np.diff(starts_p).astype(F32)
    cnt_f = np.diff(starts_f).astype(F32)
    cnt_s = np.diff(starts_s).astype(F32)

    # (dst array, node-type starts) per edge type; dst node spaces:
    # pf->file, fp->proc, ps->sock, sp->proc
    types = [
        (inp["ei_pf_dst"], starts_f),
        (inp["ei_fp_dst"], starts_p),
        (inp["ei_ps_dst"], starts_s),
        (inp["ei_sp_dst"], starts_p),
    ]

    # Per-type segment offset = max batch size (even); table = 2 segments.
    # fp and sp share the proc node space so they share one offset (stage-2
    # joint matmul needs column-aligned fp/sp presence rows).
    def _even(x):
        return int(x) + int(x) % 2

    off_f = _even(cnt_f.max())
    off_p = _even(cnt_p.max())
    off_s = _even(cnt_s.max())
    offs = [off_f, off_p, off_s, off_p]

    # Each 2-segment table must fit the local_scatter limit (n*32 < 2^16).
    # Statistically certain for the stated generator; otherwise fall back to
    # a host implementation so correctness is never at risk.
    ok = all(2 * o <= TBL for o in offs)
    if not ok or os.environ.get("KERNEL_HOST_FALLBACK"):
        m_pf, c_pf = _host_counts(inp["ei_pf_dst"], inp["batch_file"], N_FILE)
        m_fp, c_fp = _host_counts(inp["ei_fp_dst"], inp["batch_proc"], N_PROC)
        m_ps, c_ps = _host_counts(inp["ei_ps_dst"], inp["batch_sock"], N_SOCK)
        m_sp, c_sp = _host_counts(inp["ei_sp_dst"], inp["batch_proc"], N_PROC)
        c_11 = np.bincount(inp["batch_proc"], weights=m_fp * m_sp,
                           minlength=BSZ).astype(F32)
        return _epilogue(inp, c_pf, c_fp, c_ps, c_sp, c_11, cnt_p, cnt_f, cnt_s)

    routed = []
    Ks = []
    for (dst, s), o in zip(types, offs):
        arr, K = _route_edges(dst, s, o)
        routed.append(arr)
        Ks.append(K)

    # wmat: group one-hot (partition p -> out row p//16); wmat2 folds
    # fp(32+g) + sp(96+g) onto partition g for the joint AND.
    parts = np.arange(128)
    wmat = np.zeros((128, 8), bf16)
    wmat[parts, parts // 16] = 1.0
    wmat2 = np.zeros((128, 128), bf16)
    g8 = np.arange(NGRP)
    wmat2[32 + g8, g8] = 1.0
    wmat2[96 + g8, g8] = 1.0

    in_maps = []
    for c in range(NCORE):
        edges = np.concatenate(
            [routed[t][8 * c : 8 * c + 8].reshape(128, Ks[t]) for t in range(4)],
            axis=1,
        )
        in_maps.append({
            "edges": np.ascontiguousarray(edges), "wmat": wmat, "wmat2": wmat2,
        })

    nc = _build_program(Ks, offs)
    from concourse.bass_utils import run_bass_kernel_spmd

    try:
        res = run_bass_kernel_spmd(
            nc, in_maps, core_ids=list(range(NCORE)),
            trace=bool(os.environ.get("KERNEL_TRACE")),
        )
    except ModuleNotFoundError:
        res = run_bass_kernel_spmd(
            nc, in_maps, core_ids=list(range(NCORE)), trace=False
        )
    if os.environ.get("KERNEL_TRACE"):
        kernel.last_results = res

    # Decode per-(type,group) counts back to per-batch counts
    c_arr = np.zeros((5, BSZ), F32)  # pf, fp, ps, sp, joint
    for c in range(NCORE):
        v = res.results[c]["counts"]  # [128, 4] f32
        for g in range(NGRP):
            b0 = BPC * c + 2 * g
            for s in range(2):
                for t in range(4):
                    c_arr[t, b0 + s] = v[32 * t + g, s]
                c_arr[4, b0 + s] = v[g, 2 + s]
    return _epilogue(inp, c_arr[0], c_arr[1], c_arr[2], c_arr[3], c_arr[4],
                     cnt_p, cnt_f, cnt_s)


# revision 13
# speedup vs baseline: 1.2320x; 1.0214x over previous
"""Trainium2 Bass kernel for nn_HANGraphClassifier.

Because every node of a type shares one embedding, the GAT attention collapses
analytically: per-edge softmax weights become 1/deg and each dst node's
aggregated message is src_type_vec * (in_degree > 0). The whole forward pass
therefore reduces to per-batch counts of dst nodes with >=1 incoming edge
(per edge type, plus the joint fp&sp combination for proc nodes), followed by
tiny [BSZ,64] parameter-only math.

Device work (the O(E)+O(N) part): presence-mask scatter over 4.8M edges and
per-batch counting, on 8 NeuronCores.

Sharding (per the hint, graph/data-parallel by destination-node partition):
 - batches 16c..16c+15 -> core c (batch arrays are sorted, so each core owns a
   contiguous dst-node range per node type).
 - within a core, Q7 group g (16 SBUF partitions) owns the node range of
   batches (16c+2g, 16c+2g+1) -- a "bucket" of ~1560 nodes (<= 2046).
 - each edge type's dst list is routed on the host into these 64 buckets and
   converted to bucket-local int16 indices (standard global->local id
   conversion during partitioning); a bucket's edges are split arbitrarily
   across its 16 partitions.

Device program per core (single SPMD program, ~30 instructions):
 1. DMA the routed [128, Ktot] int16 index array in.
 2. gpsimd.local_scatter per edge type: each partition scatters 1.0 into its
    own [2046] bf16 table copy (SuperGather HW; duplicates all write 1.0).
 3. PE matmul with a [128->8] group-indicator weight: sums the 16 copies of
    each group -> PSUM [32, 2046] per-(type,group) copy-counts.
 4. DVE: presence = min(count,1); joint = min(pres_fp, pres_sp);
    multiply by a host-built segment mask (1.0 for the bucket's first batch,
    4096.0 for the second) and reduce -> [40,1] encoded per-batch counts.
 5. DMA counts out; host decodes c0 = v % 4096, c1 = v // 4096.
"""

import os

import numpy as np

N_PROC, N_FILE, N_SOCK = 100000, 100000, 50000
H, D, HID, BSZ, NCLS = 4, 16, 64, 128, 2
NCORE = 8
BPC = BSZ // NCORE          # batches per core = 16
NGRP = 8                    # Q7 groups per core
TBL = 2046                  # local_scatter table entries (limit: n*32 < 2^16)
NROW = 40                   # 4 types * 8 groups + 8 joint rows
F32 = np.float32


def _batch_starts(batch, n_nodes):
    s = np.searchsorted(batch, np.arange(BSZ + 1)).astype(np.int64)
    assert s[-1] == n_nodes
    return s


def _route_edges(dst, starts, seg_off):
    """Route one edge type's dst list into 64 batch-pair buckets; local index
    = dst - batch_start, with the bucket's second batch placed at column
    seg_off so per-batch counts fall out of a fixed-stride reduce.

    Returns ([64,16,K] int16 local idx array padded with -1, K)."""
    bid = (np.searchsorted(starts, dst, side="right") - 1).astype(np.int32)
    order = np.argsort(bid, kind="stable")
    sd = dst[order]
    sb = bid[order]
    loc = (sd - starts[sb] + (sb & 1) * seg_off).astype(np.int16)
    cnts = np.bincount(bid >> 1, minlength=64)
    per_part = (cnts + 15) // 16
    K = int(max(2, per_part.max()))
    K += K % 2  # num_idxs must be even
    arr = np.full((64, 16 * K), -1, np.int16)
    off = np.concatenate([[0], np.cumsum(cnts)])
    for k in range(64):
        if cnts[k]:
            arr[k, : cnts[k]] = loc[off[k] : off[k] + cnts[k]]
    return arr.reshape(64, 16, K), K


def _host_counts(dst, batch, n_nodes):
    m = np.zeros(n_nodes, F32)
    m[dst] = 1.0
    return m, np.bincount(batch, weights=m, minlength=BSZ).astype(F32)


def _epilogue(inp, c_pf, c_fp, c_ps, c_sp, c_11, cnt_p, cnt_f, cnt_s):
    """Tiny parameter-only math reproducing the collapsed reference."""
    node_emb, proj_w, proj_b = inp["node_emb"], inp["proj_w"], inp["proj_b"]
    k_w, k_b, q_vec = inp["k_w"], inp["k_b"], inp["q_vec"]
    p = [node_emb[i] @ proj_w[i].T + proj_b[i] for i in range(3)]
    rp = [np.maximum(v, 0).astype(F32) for v in p]

    def score(v, n1, N):
        t1 = np.tanh(v @ k_w.T + k_b)
        t0 = np.tanh(k_b)
        mean = (n1 * t1 + (N - n1) * t0) / F32(N)
        return (q_vec * mean).sum()

    s1 = score(rp[1], c_fp.sum(), N_PROC)
    s2 = score(rp[2], c_sp.sum(), N_PROC)
    e = np.exp(np.array([s1, s2]) - max(s1, s2))
    attn = (e / e.sum()).astype(F32)

    h10 = np.maximum(attn[0] * rp[1], 0)
    h01 = np.maximum(attn[1] * rp[2], 0)
    h11 = np.maximum(attn[0] * rp[1] + attn[1] * rp[2], 0)

    c_10, c_01 = c_fp - c_11, c_sp - c_11
    pool_p = (np.outer(c_10, h10) + np.outer(c_01, h01) + np.outer(c_11, h11)) \
        / np.maximum(cnt_p, 1.0)[:, None]
    pool_f = np.outer(c_pf, rp[0]) / np.maximum(cnt_f, 1.0)[:, None]
    pool_s = np.outer(c_ps, rp[0]) / np.maximum(cnt_s, 1.0)[:, None]
    g = ((pool_p + pool_f + pool_s) / 3.0).astype(F32)
    h = np.maximum(g @ inp["cls_w1"].T + inp["cls_b1"], 0)
    return (h @ inp["cls_w2"].T + inp["cls_b2"]).astype(F32)


_PROG_CACHE = {}


def _build_program(Ks, offs):
    import concourse.bacc as bacc
    import concourse.mybir as mybir
    import concourse.tile as tile

    key = (tuple(Ks), tuple(offs))
    if key in _PROG_CACHE:
        return _PROG_CACHE[key]

    Ktot = sum(Ks)
    Kmax = max(Ks)
    elems = [2 * o for o in offs]         # per-type table size (2 segments)
    ecol = np.concatenate([[0], np.cumsum(elems)]).astype(int)
    emax = max(elems)
    ep = elems[1]                          # proc table width (fp & sp share)
    nc = bacc.Bacc("TRN2", target_bir_lowering=False, debug=False)
    ed_d = nc.dram_tensor("edges", [128, Ktot], mybir.dt.int16, kind="ExternalInput")
    wm_d = nc.dram_tensor("wmat", [128, 8], mybir.dt.bfloat16, kind="ExternalInput")
    w2_d = nc.dram_tensor("wmat2", [128, 128], mybir.dt.bfloat16, kind="ExternalInput")
    ct_d = nc.dram_tensor("counts", [128, 4], mybir.dt.float32, kind="ExternalOutput")

    with tile.TileContext(nc, trace_sim=False) as tc:
        with (
            tc.tile_pool(name="sb", bufs=1) as pool,
            tc.tile_pool(name="ps", bufs=1, space="PSUM") as ppool,
        ):
            ed = pool.tile([128, Ktot], mybir.dt.int16)
            wm = pool.tile([128, 8], mybir.dt.bfloat16)
            w2 = pool.tile([128, 128], mybir.dt.bfloat16)
            ones = pool.tile([128, Kmax], mybir.dt.bfloat16)
            tbl = pool.tile([128, int(ecol[4])], mybir.dt.bfloat16)
            pres = pool.tile([128, emax], mybir.dt.bfloat16)
            pres2 = pool.tile([128, ep], mybir.dt.bfloat16)
            red = pool.tile([128, 4], mybir.dt.float32)
            ps = ppool.tile([128, emax], mybir.dt.float32)
            ps2 = ppool.tile([128, ep], mybir.dt.float32)

            dum_i = pool.tile([128, 2], mybir.dt.int16)
            dum_d = pool.tile([128, 2], mybir.dt.bfloat16)
            dum_o = pool.tile([128, 2], mybir.dt.bfloat16)

            nc.sync.dma_start(ed[:], ed_d[:])
            nc.sync.dma_start(wm[:], wm_d[:])
            nc.sync.dma_start(w2[:], w2_d[:])
            nc.vector.memset(dum_i[:], -1)
            nc.vector.memset(dum_d[:], 0.0)
            nc.vector.memset(ones[:], 1.0)
            # stage-2 contracts over all 128 pres partitions; unused rows
            # must be 0.0, not stale SBUF (0 * NaN would poison PSUM)
            nc.vector.memset(pres[:], 0.0)

            # warmup scatter: forces the ~6us ext-isa IRAM load to overlap
            # the entry barrier + edge DMA instead of gating the real work
            nc.gpsimd.local_scatter(
                dum_o[:], dum_d[:], dum_i[:],
                channels=128, num_elems=2, num_idxs=2,
            )

            ofs_tbl = [0, Ks[0], Ks[0] + Ks[1], Ks[0] + Ks[1] + Ks[2]]
            # smallest type (ps/sock) last: its short min+reduce tail, and the
            # joint chain runs under its scatter
            for t in (0, 1, 3, 2):
                ofs = ofs_tbl[t]
                e0, e1 = int(ecol[t]), int(ecol[t + 1])
                nc.gpsimd.local_scatter(
                    tbl[:, e0:e1],
                    ones[:, : Ks[t]],
                    ed[:, ofs : ofs + Ks[t]],
                    channels=128,
                    num_elems=elems[t],
                    num_idxs=Ks[t],
                )
                # per-(type,group) copy-count sums land at partitions
                # 32t+g via explicit PE tile position; presence + per-batch
                # reduce for this type overlap the next type's scatter.
                for j0 in range(0, elems[t], 512):
                    j1 = min(j0 + 512, elems[t])
                    nc.tensor.matmul(
                        out=ps[32 * t : 32 * t + 8, j0:j1],
                        lhsT=wm[:, 0:8],
                        rhs=tbl[:, e0 + j0 : e0 + j1],
                        start=True,
                        stop=True,
                        tile_position=(0, 32 * t),
                    )
                nc.vector.tensor_scalar(
                    pres[32 * t : 32 * t + 8, : elems[t]],
                    ps[32 * t : 32 * t + 8, : elems[t]],
                    1.0, None, op0=mybir.AluOpType.min,
                )
                nc.vector.tensor_reduce(
                    out=red[32 * t : 32 * t + 8, 0:2],
                    in_=pres[32 * t : 32 * t + 8, : elems[t]].rearrange(
                        "p (s o) -> p s o", s=2
                    ),
                    axis=mybir.AxisListType.X,
                    op=mybir.AluOpType.add,
                )
                if t == 3:
                    # joint fp&sp: re-align fp (rows 32..39) and sp (rows
                    # 96..103) onto partitions 0..7 by summing; sum-1
                    # clamped at 0 is the AND. Runs under the ps scatter.
                    for j0 in range(0, ep, 512):
                        j1 = min(j0 + 512, ep)
                        nc.tensor.matmul(
                            out=ps2[:, j0:j1],
                            lhsT=w2[:],
                            rhs=pres[:, j0:j1],
                            start=True,
                            stop=True,
                        )
                    nc.vector.tensor_scalar(
                        pres2[:], ps2[:], 1.0, 0.0,
                        op0=mybir.AluOpType.subtract, op1=mybir.AluOpType.max,
                    )
                    nc.vector.tensor_reduce(
                        out=red[0:8, 2:4],
                        in_=pres2[0:8, :].rearrange("p (s o) -> p s o", s=2),
                        axis=mybir.AxisListType.X,
                        op=mybir.AluOpType.add,
                    )
            nc.sync.dma_start(ct_d[:], red[:])

    nc.compile()
    _PROG_CACHE[key] = nc
    return nc


def kernel(**inputs):
    import ml_dtypes

    inp = {k: np.asarray(v) for k, v in inputs.items()}
    bf16 = ml_dtypes.bfloat16

    starts_p = _batch_starts(inp["batch_proc"], N_PROC)
    starts_f = _batch_starts(inp["batch_file"], N_FILE)
    starts_s = _batch_starts(inp["batch_sock"], N_SOCK)
    cnt_p = np.diff(starts_p).astype(F32)
    cnt_f = np.diff(starts_f).astype(F32)
    cnt_s = np.diff(starts_s).astype(F32)

    # (dst array, node-type starts) per edge type; dst node spaces:
    # pf->file, fp->proc, ps->sock, sp->proc
    types = [
        (inp["ei_pf_dst"], starts_f),
        (inp["ei_fp_dst"], starts_p),
        (inp["ei_ps_dst"], starts_s),
        (inp["ei_sp_dst"], starts_p),
    ]

    # Per-type segment offset = max batch size (even); table = 2 segments.
    # fp and sp share the proc node space so they share one offset (stage-2
    # joint matmul needs column-aligned fp/sp presence rows).
    def _even(x):
        return int(x) + int(x) % 2

    off_f = _even(cnt_f.max())
    off_p = _even(cnt_p.max())
    off_s = _even(cnt_s.max())
    offs = [off_f, off_p, off_s, off_p]

    # Each 2-segment table must fit the local_scatter limit (n*32 < 2^16).
    # Statistically certain for the stated generator; otherwise fall back to
    # a host implementation so correctness is never at risk.
    ok = all(2 * o <= TBL for o in offs)
    if not ok or os.environ.get("KERNEL_HOST_FALLBACK"):
        m_pf, c_pf = _host_counts(inp["ei_pf_dst"], inp["batch_file"], N_FILE)
        m_fp, c_fp = _host_counts(inp["ei_fp_dst"], inp["batch_proc"], N_PROC)
        m_ps, c_ps = _host_counts(inp["ei_ps_dst"], inp["batch_sock"], N_SOCK)
        m_sp, c_sp = _host_counts(inp["ei_sp_dst"], inp["batch_proc"], N_PROC)
        c_11 = np.bincount(inp["batch_proc"], weights=m_fp * m_sp,
                           minlength=BSZ).astype(F32)
        return _epilogue(inp, c_pf, c_fp, c_ps, c_sp, c_11, cnt_p, cnt_f, cnt_s)

    routed = []
    Ks = []
    for (dst, s), o in zip(types, offs):
        arr, K = _route_edges(dst, s, o)
        routed.append(arr)
        Ks.append(K)

    # wmat: group one-hot (partition p -> out row p//16); wmat2 folds
    # fp(32+g) + sp(96+g) onto partition g for the joint AND.
    parts = np.arange(128)
    wmat = np.zeros((128, 8), bf16)
    wmat[parts, parts // 16] = 1.0
    wmat2 = np.zeros((128, 128), bf16)
    g8 = np.arange(NGRP)
    wmat2[32 + g8, g8] = 1.0
    wmat2[96 + g8, g8] = 1.0

    in_maps = []
    for c in range(NCORE):
        edges = np.concatenate(
            [routed[t][8 * c : 8 * c + 8].reshape(128, Ks[t]) for t in range(4)],
            axis=1,
        )
        in_maps.append({
            "edges": np.ascontiguousarray(edges), "wmat": wmat, "wmat2": wmat2,
        })

    nc = _build_program(Ks, offs)
    from concourse.bass_utils import run_bass_kernel_spmd

    try:
        res = run_bass_kernel_spmd(
            nc, in_maps, core_ids=list(range(NCORE)),
            trace=bool(os.environ.get("KERNEL_TRACE")),
        )
    except ModuleNotFoundError:
        res = run_bass_kernel_spmd(
            nc, in_maps, core_ids=list(range(NCORE)), trace=False
        )
    if os.environ.get("KERNEL_TRACE"):
        kernel.last_results = res

    # Decode per-(type,group) counts back to per-batch counts
    c_arr = np.zeros((5, BSZ), F32)  # pf, fp, ps, sp, joint
    for c in range(NCORE):
        v = res.results[c]["counts"]  # [128, 4] f32
        for g in range(NGRP):
            b0 = BPC * c + 2 * g
            for s in range(2):
                for t in range(4):
                    c_arr[t, b0 + s] = v[32 * t + g, s]
                c_arr[4, b0 + s] = v[g, 2 + s]
    return _epilogue(inp, c_arr[0], c_arr[1], c_arr[2], c_arr[3], c_arr[4],
                     cnt_p, cnt_f, cnt_s)
